# revision 11
# baseline (speedup 1.0000x reference)
"""Trainium2 Bass kernel for nn_AttentionBlock (GroupNorm + MHA + proj + residual).

Problem: x[8, 512, 32, 32] fp32; GroupNorm(32 groups) -> qkv (1x1 conv) ->
8-head attention over 1024 spatial positions -> proj -> residual.

Sharding: data-parallel over batch. 8 batch elements -> 8 NeuronCores,
one SPMD NEFF. No collectives.

v4 design (per-core, x as [c=512, n=1024]):
  - ACT is the pacer: 64 exp chunks of [128,1024] (~67 us). All other
    ACT work moved to DVE/Pool; ALL 32 of j=0/j=1's exps start during
    the qkv phase (fp8 probs pair tiles keep 16 pairs resident).
  - PE moving-element rate is dtype-independent on this hw, so no fp8
    matmul tricks; probs/vT are fp8e4 only to shrink SBUF (probs are
    exp(s-6); the shift cancels in the softmax normalization and keeps
    exp inside fp8e4 range). AV accumulates f32 in psum, v2-shaped.
  - softmax normalization is fully on-chip (v3's DRAM round-trip
    broadcast crawled at ~2GB/s on the critical tail): denominators ->
    dd via sbuf-sbuf DMA reshape -> DVE reciprocal [128,2,8] -> rrT
    [2,N] via sbuf-sbuf DMA -> PE broadcast matmul with a 0/1
    selection lhsT [2,128] -> rb psum -> one DVE mul per j.
  - weights DMA'd in use order (q-m0, k-m0, v, q/k m1-3, proj) so the
    first score matmuls start ~15us earlier; xh standardization split
    DVE/Pool to halve its serial cost.
  - proj: kc 0-2 for all m first, then kc=3 per-m with eviction+DMA
    pipelined; y output in bf16 (halves writeback, error budget ok).
  - GroupNorm: selection-matrix matmuls for group reduce, rsqrt via
    exp(-0.5*ln(var+eps)); per-channel affine folded into qkv weights.
"""
import sys

sys.path.insert(0, "/opt/trn_rl_repo")

import numpy as np

import concourse.bass as bass
import concourse.bacc as bacc
import concourse.tile as tile
from concourse import mybir
from concourse.bass_utils import run_bass_kernel_spmd

F32 = mybir.dt.float32
F32R = mybir.dt.float32r
BF16 = mybir.dt.bfloat16
F8 = mybir.dt.float8e4
AX = mybir.AxisListType
OP = mybir.AluOpType
AF = mybir.ActivationFunctionType

C = 512          # channels
N = 1024         # spatial positions (32*32)
HEADS = 8
HD = 64          # head dim
G = 32           # groups
GSZ = 16         # channels per group
EPS = 1e-6
NC4 = 4          # channel chunks of 128
NM8 = 8          # spatial chunks of 128
EXP_SHIFT = -6.0  # exp(s-6): cancels in softmax, keeps exp in fp8e4 range
WQ0 = 0          # wpack col offsets
WK0 = 512
WV0 = 1024
WP0 = 1536
WPACK_COLS = 2048


def build_nc():
    nc = bacc.Bacc(None)
    x = nc.declare_dram_parameter("x", [C, N], F32, isOutput=False)
    wpack = nc.declare_dram_parameter("wpack", [C, WPACK_COLS], BF16, isOutput=False)
    gsel = nc.declare_dram_parameter("gsel", [C, G], F32, isOutput=False)
    gselT = nc.declare_dram_parameter("gselT", [G, C], F32, isOutput=False)
    bpack = nc.declare_dram_parameter("bpack", [128, 12], F32, isOutput=False)
    sel2d = nc.declare_dram_parameter("sel2d", [2, 128], F32, isOutput=False)
    y = nc.declare_dram_parameter("y", [C, N], BF16, isOutput=True)

    with tile.TileContext(nc) as tc:
        with (
            tc.tile_pool(name="const", bufs=1) as const,
            tc.tile_pool(name="main", bufs=1) as main,
        ):
            # ---- Phase 0: input DMAs ----
            # x chunks on the sync queue; weight column-groups on the scalar
            # queue in USE order so q/k m=0 land first.
            x_sb = main.tile([128, NC4, N], F32)
            xv = x[:].rearrange("(c p) n -> p c n", p=128)
            for c4 in range(NC4):
                nc.sync.dma_start(x_sb[:, c4, :], xv[:, c4, :])
            gs_sb = const.tile([128, NC4, G], F32)
            nc.sync.dma_start(gs_sb[:], gsel[:].rearrange("(c p) g -> p c g", p=128))
            gt_sb = const.tile([G, C], F32)
            nc.sync.dma_start(gt_sb[:], gselT[:])
            bp_sb = const.tile([128, 12], F32)
            nc.sync.dma_start(bp_sb[:], bpack[:])
            wp_sb = const.tile([128, NC4, WPACK_COLS], BF16)
            wpv = wpack[:].rearrange("(c p) m -> p c m", p=128)
            # use-order column groups: q-m0, k-m0, v, q-m123, k-m123, proj
            for lo, hi in ((WQ0, WQ0 + 128), (WK0, WK0 + 128),
                           (WV0, WV0 + C), (WQ0 + 128, WQ0 + 512),
                           (WK0 + 128, WK0 + 512), (WP0, WP0 + C)):
                nc.scalar.dma_start(wp_sb[:, :, lo:hi], wpv[:, :, lo:hi])

            # ---- Phase 1: GroupNorm stats -> xh (standardized x, bf16) ----
            cst = const.tile([128, NC4, 2], F32)       # per-channel sum | sumsq
            gsb = const.tile([G, 8], F32)              # group scratch
            eps_t = const.tile([G, 1], F32)
            nc.vector.memset(eps_t[:], EPS)
            shift_t = const.tile([128, 1], F32)        # exp bias (-6)
            nc.vector.memset(shift_t[:], EXP_SHIFT)
            sel2 = const.tile([2, 128], F32)           # hh-broadcast selector
            nc.sync.dma_start(sel2[:], sel2d[:])
            gstats = const.tile([G, 2], F32)           # rs | -mean*rs
            chsc = const.tile([128, NC4, 2], F32)      # per-channel rs | bias
            xh_sb = main.tile([128, NC4, N], BF16)

            # prefire the Square table set first (the stats need it next);
            # the Ln/Exp set is loaded once after the squares finish
            nc.scalar.activation(gsb[:, 4:5], eps_t[:], AF.Square)

            with (
                tc.tile_pool(name="sq", bufs=2) as sqp,
                tc.tile_pool(name="pst", bufs=2, space="PSUM") as pst,
            ):
                for c4 in range(NC4):
                    nc.vector.tensor_reduce(
                        cst[:, c4, 0:1], x_sb[:, c4, :], axis=AX.X, op=OP.add
                    )
                    sq = sqp.tile([128, N], F32)
                    nc.scalar.activation(
                        sq[:], x_sb[:, c4, :], AF.Square,
                        accum_out=cst[:, c4, 1:2],
                    )
                gs_ps = pst.tile([G, 2], F32)
                for c4 in range(NC4):
                    nc.tensor.matmul(
                        gs_ps[:],
                        gs_sb[:, c4, :],
                        cst[:, c4, :],
                        start=(c4 == 0),
                        stop=(c4 == NC4 - 1),
                    )
                nc.vector.tensor_copy(gsb[:, 0:2], gs_ps[:])
                # mean = gsb[:,0], ex2 = gsb[:,1] (both already / 16384)
                nc.vector.tensor_mul(gsb[:, 2:3], gsb[:, 0:1], gsb[:, 0:1])
                nc.vector.tensor_sub(gsb[:, 3:4], gsb[:, 1:2], gsb[:, 2:3])
                nc.scalar.activation(gsb[:, 4:5], gsb[:, 3:4], AF.Ln, bias=eps_t[:])
                nc.scalar.activation(gstats[:, 0:1], gsb[:, 4:5], AF.Exp, scale=-0.5)
                nc.vector.tensor_mul(gsb[:, 6:7], gsb[:, 0:1], gstats[:, 0:1])
                nc.vector.tensor_scalar_mul(gstats[:, 1:2], gsb[:, 6:7], -1.0)
                for c4 in range(NC4):
                    cs_ps = pst.tile([128, 2], F32)
                    nc.tensor.matmul(
                        cs_ps[:],
                        gt_sb[:, c4 * 128:(c4 + 1) * 128],
                        gstats[:],
                        start=True,
                        stop=True,
                    )
                    nc.vector.tensor_copy(chsc[:, c4, :], cs_ps[:])
                # xh standardization split DVE (c4 0,1) / Pool (c4 2,3) to
                # halve the serial chain ahead of the first qkv matmuls
                for c4 in range(NC4):
                    eng = nc.vector if c4 < 2 else nc.gpsimd
                    eng.tensor_scalar(
                        xh_sb[:, c4, :], x_sb[:, c4, :],
                        chsc[:, c4, 0:1], chsc[:, c4, 1:2],
                        op0=OP.mult, op1=OP.add,
                    )

            # ---- Phase 2: qkv ----
            q_sb = main.tile([128, NC4, N], BF16)
            k_sb = main.tile([128, NC4, N], BF16)
            vt_sb = main.tile([128, NM8, HEADS, HD + 1], F8)
            nc.vector.memset(vt_sb[:, :, :, HD:HD + 1], 1.0)

            # probs pool lives across phase 2 + 3: ALL of j=0 and j=1's exp
            # chunks run DURING the qkv phase (ACT is otherwise idle there
            # while attention is ACT-bound). Tiles are kq-PAIRS [128, 2, N]
            # in fp8e4.
            probs_pool = tc.tile_pool(name="probs", bufs=20)
            probs = probs_pool.__enter__()
            ptE = {}
            rows = (slice(0, HD), slice(HD, 128))

            with (
                tc.tile_pool(name="pmm", bufs=1, space="PSUM") as pmm,
                tc.tile_pool(name="pvv", bufs=2, space="PSUM") as pvv,
                tc.tile_pool(name="pesc", bufs=2, space="PSUM") as pesc,
            ):
                # q/k m=0 first: j=0's scores only need these
                for dst, woff, boff in ((q_sb, WQ0, 0), (k_sb, WK0, 4)):
                    ps = pmm.tile([128, N], F32, tag="qk")
                    for kc in range(NC4):
                        for nh in range(2):
                            nc.tensor.matmul(
                                ps[:, nh * 512:(nh + 1) * 512],
                                wp_sb[:, kc, woff:woff + 128],
                                xh_sb[:, kc, nh * 512:(nh + 1) * 512],
                                start=(kc == 0),
                                stop=(kc == NC4 - 1),
                            )
                    nc.vector.tensor_scalar_add(
                        dst[:, 0, :], ps[:], bp_sb[:, boff:boff + 1]
                    )
                # vT: stationary xh spatial chunk, moving all v weights
                for mt in range(NM8):
                    ps = pvv.tile([128, 512], F32, tag="v")
                    for kc in range(NC4):
                        nc.tensor.matmul(
                            ps[:],
                            xh_sb[:, kc, mt * 128:(mt + 1) * 128],
                            wp_sb[:, kc, WV0:WV0 + C],
                            start=(kc == 0),
                            stop=(kc == NC4 - 1),
                        )
                    nc.vector.tensor_copy(
                        vt_sb[:, mt, :, 0:HD],
                        ps[:].rearrange("p (h c) -> p h c", h=HEADS),
                    )

                # q/k m=1..3 emitted one matmul at a time between the early
                # score chunks below, so the PE stays dense while ACT exps
                fill = []

                def qk_unit(dst, woff, boff, m):
                    ps = pmm.tile([128, N], F32, name=f"qk{woff}_{m}", tag="qk")
                    for kc in range(NC4):
                        for nh in range(2):
                            fill.append(lambda ps=ps, kc=kc, nh=nh, woff=woff, m=m: nc.tensor.matmul(
                                ps[:, nh * 512:(nh + 1) * 512],
                                wp_sb[:, kc, woff + m * 128:woff + (m + 1) * 128],
                                xh_sb[:, kc, nh * 512:(nh + 1) * 512],
                                start=(kc == 0),
                                stop=(kc == NC4 - 1),
                            ))
                    fill.append(lambda ps=ps, dst=dst, m=m, boff=boff: nc.vector.tensor_scalar_add(
                        dst[:, m, :], ps[:], bp_sb[:, boff + m:boff + m + 1]
                    ))

                for m in range(1, 4):
                    qk_unit(q_sb, WQ0, 0, m)
                    qk_unit(k_sb, WK0, 4, m)

                # early scores+exp for ALL of j=0 and j=1 (32 chunks),
                # interleaved with the remaining qkv matmuls
                for c in range(32):
                    j01, kq, hh = c // 16, (c % 16) // 2, c % 2
                    esc = pesc.tile([128, N], F32, name=f"esc{c}", tag="esc")
                    for nh in range(2):
                        nc.tensor.matmul(
                            esc[:, nh * 512:(nh + 1) * 512],
                            k_sb[rows[hh], j01, kq * 128:(kq + 1) * 128],
                            q_sb[rows[hh], j01, nh * 512:(nh + 1) * 512],
                            start=True,
                            stop=True,
                        )
                    key = (j01, kq // 2, hh)
                    if kq % 2 == 0:
                        ptp = probs.tile([128, 2, N], F8, name=f"ptE{key}", tag="pt")
                        ptE[key] = ptp
                    else:
                        ptp = ptE[key]
                    nc.scalar.activation(
                        ptp[:, kq % 2, :], esc[:], AF.Exp, bias=shift_t[:]
                    )
                    for _ in range(3):
                        if fill:
                            fill.pop(0)()
                while fill:
                    fill.pop(0)()

            # ---- Phase 3: attention + proj ----
            # Part A (jj=0,1): each chunk emits scores+exp for j=jj+2 and
            # av matmuls for j=jj (one chunk behind), so PE duty stays high
            # through the exp-paced stretch and ALL exps finish by the end
            # of jj=1. Part B: dense av j2/j3 + proj, PE back-to-back.
            aun_ch = main.tile([128, NC4, N], F32)      # unnormalized A
            a_sb = main.tile([128, NC4, N], BF16)       # normalized A

            with (
                tc.tile_pool(name="spool", bufs=2) as spool,
                tc.tile_pool(name="dpool", bufs=2) as dpool,
                tc.tile_pool(name="psc", bufs=2, space="PSUM") as psc,
                tc.tile_pool(name="pav", bufs=1, space="PSUM") as pav,
            ):
                pending = []

                def flush():
                    while pending:
                        pending.pop(0)()

                def mk_av(j, kq, hh, ptp, av):
                    def go():
                        for nh in range(2):
                            nc.tensor.matmul(
                                av[:, hh, nh * 512:(nh + 1) * 512],
                                vt_sb[:, kq, 2 * j + hh, :],
                                ptp[:, kq % 2, nh * 512:(nh + 1) * 512],
                                start=(kq == 0),
                                stop=(kq == NM8 - 1),
                            )
                    return go

                def evict_normalize(j, av, act_hh0):
                    # evict av rows, then fully on-chip softmax
                    # normalization (no DRAM round trip):
                    #   dd[128,2,8] <- denominator rows (sbuf-sbuf DMA)
                    #   rr = 1/dd (DVE); rrT[2,N] <- rr (sbuf-sbuf DMA)
                    #   rb[128,N] = sel2.T @ rrT (PE broadcast into psum)
                    #   a_sb = aun * rb (DVE)
                    dd = dpool.tile([128, 2, 8], F32, name=f"dd{j}", tag="dd")
                    for hh in range(2):
                        s_t = spool.tile(
                            [HD + 1, N], F32, name=f"st{j}_{hh}", tag=f"st{hh}"
                        )
                        if act_hh0 and hh == 0:
                            # ACT is exp-idle here: run the two eviction
                            # copies on ACT+DVE in parallel
                            nc.scalar.activation(s_t[:], av[:, hh, :], AF.Copy)
                        else:
                            nc.vector.tensor_copy(s_t[:], av[:, hh, :])
                        nc.sync.dma_start(dd[:, hh, :], s_t[HD:HD + 1, :])
                        nc.sync.dma_start(aun_ch[rows[hh], j, :], s_t[0:HD, :])
                    rr = dpool.tile([128, 2, 8], F32, name=f"rr{j}", tag="rr")
                    nc.vector.reciprocal(rr[:], dd[:])
                    rrT = dpool.tile([2, N], F32, name=f"rrT{j}", tag="rrT")
                    for hh in range(2):
                        nc.sync.dma_start(rrT[hh:hh + 1, :], rr[:, hh, :])
                    rb = pav.tile([128, N], F32, name=f"rb{j}", tag="av")
                    for nh in range(2):
                        nc.tensor.matmul(
                            rb[:, nh * 512:(nh + 1) * 512],
                            sel2[:],
                            rrT[:, nh * 512:(nh + 1) * 512],
                            start=True,
                            stop=True,
                        )
                    nc.vector.tensor_mul(a_sb[:, j, :], aun_ch[:, j, :], rb[:])

                # ---- Part A ----
                for jj in range(2):
                    av = pav.tile([HD + 1, 2, N], F32, name=f"av{jj}", tag="av")
                    j2 = jj + 2
                    for kq in range(NM8):
                        for hh in range(2):
                            sps = psc.tile(
                                [128, N], F32, name=f"sps{j2}_{kq}_{hh}",
                                tag="sps",
                            )
                            # small warmth keeper every other chunk keeps
                            # the PE HAM window busy during exp-paced code
                            if (kq + hh) % 2 == 0:
                                nc.tensor.matmul(
                                    sps[:, 0:128],
                                    wp_sb[:, 0, 0:128],
                                    xh_sb[:, 0, 0:128],
                                    start=True,
                                    stop=True,
                                )
                            for nh in range(2):
                                nc.tensor.matmul(
                                    sps[:, nh * 512:(nh + 1) * 512],
                                    k_sb[rows[hh], j2, kq * 128:(kq + 1) * 128],
                                    q_sb[rows[hh], j2, nh * 512:(nh + 1) * 512],
                                    start=True,
                                    stop=True,
                                )
                            key2 = (j2, kq // 2, hh)
                            if kq % 2 == 0:
                                ptp2 = probs.tile(
                                    [128, 2, N], F8,
                                    name=f"pt{j2}_{kq // 2}_{hh}", tag="pt",
                                )
                                ptE[key2] = ptp2
                            else:
                                ptp2 = ptE[key2]
                            nc.scalar.activation(
                                ptp2[:, kq % 2, :], sps[:], AF.Exp,
                                bias=shift_t[:],
                            )
                            pending.append(
                                mk_av(jj, kq, hh, ptE[(jj, kq // 2, hh)], av)
                            )
                            if len(pending) > 1:
                                pending.pop(0)()
                    flush()
                    evict_normalize(jj, av, act_hh0=False)

                # ---- Part B: dense av j2/j3 + proj ----
                yv = y[:].rearrange("(m p) n -> m p n", p=128)

                def proj_mm(ps, mi, m, kc, stop):
                    for nh in range(2):
                        nc.tensor.matmul(
                            ps[:, mi, nh * 512:(nh + 1) * 512] if ps.shape[1] == 2
                            else ps[:, nh * 512:(nh + 1) * 512],
                            wp_sb[:, kc, WP0 + m * 128:WP0 + (m + 1) * 128],
                            a_sb[:, kc, nh * 512:(nh + 1) * 512],
                            start=(kc == 0),
                            stop=stop,
                        )

                for jj in (2, 3):
                    av = pav.tile([HD + 1, 2, N], F32, name=f"av{jj}", tag="av")
                    for kq in range(NM8):
                        for hh in range(2):
                            mk_av(jj, kq, hh, ptE[(jj, kq // 2, hh)], av)()
                    if jj == 2:
                        evict_normalize(2, av, act_hh0=True)
                        # proj m=0,1 into the freed scores slots; kc=0,1
                        # overlap j2's normalize chain
                        pA = [
                            psc.tile([128, N], F32, name=f"pjA{m}", tag="sps")
                            for m in range(2)
                        ]
                        for kc in range(2):
                            for m in range(2):
                                proj_mm(pA[m], 0, m, kc, stop=False)
                    else:
                        evict_normalize(3, av, act_hh0=True)
                for m in range(2):
                    proj_mm(pA[m], 0, m, 2, stop=False)
                pB = pav.tile([128, 2, N], F32, name="pjB", tag="av")
                for kc in range(NC4):
                    for mi, m in enumerate((2, 3)):
                        proj_mm(pB, mi, m, kc, stop=(kc == NC4 - 1))
                for m in range(2):
                    proj_mm(pA[m], 0, m, 3, stop=True)

                with tc.tile_pool(name="ypool", bufs=2) as ypool:
                    for m in range(4):
                        src = pA[m][:] if m < 2 else pB[:, m - 2, :]
                        yt = ypool.tile([128, N], BF16, tag="yt")
                        # fused eviction: (psum + bias) + residual, one DVE op
                        nc.vector.scalar_tensor_tensor(
                            yt[:], src, bp_sb[:, 8 + m:9 + m], x_sb[:, m, :],
                            op0=OP.add, op1=OP.add,
                        )
                        q = (nc.sync, nc.scalar)[m % 2]
                        q.dma_start(yv[m, :, :], yt[:])

            probs_pool.__exit__(None, None, None)

    nc.compile()
    return nc


_NC_CACHE = None


def _get_nc():
    global _NC_CACHE
    if _NC_CACHE is None:
        _NC_CACHE = build_nc()
    return _NC_CACHE


def _to_bf16(a):
    import ml_dtypes
    return np.ascontiguousarray(a, np.float32).astype(ml_dtypes.bfloat16)


def _prep_host(norm_w, norm_b, qkv_w, qkv_b, proj_w, proj_b):
    g = norm_w.astype(np.float32)
    b = norm_b.astype(np.float32)
    Wq, Wk, Wv = qkv_w[0:C], qkv_w[C:2 * C], qkv_w[2 * C:3 * C]
    bq, bk, bv = qkv_b[0:C], qkv_b[C:2 * C], qkv_b[2 * C:3 * C]
    scale = np.float32(1.0 / np.sqrt(HD))

    WqT = (scale * (Wq * g[None, :])).T
    WkT = (Wk * g[None, :]).T
    WvT = (Wv * g[None, :]).T
    bq_eff = scale * (Wq @ b + bq)
    bk_eff = Wk @ b + bk
    pb_eff = proj_w @ (Wv @ b + bv) + proj_b

    cidx = np.arange(C)
    gsel = np.zeros((C, G), np.float32)
    gsel[cidx, cidx // GSZ] = np.float32(1.0 / (GSZ * N))
    gselT = np.zeros((G, C), np.float32)
    gselT[cidx // GSZ, cidx] = 1.0

    wpack = np.concatenate([WqT, WkT, WvT, proj_w.T], axis=1).astype(np.float32)
    assert wpack.shape == (C, WPACK_COLS)
    wpack_bf16 = _to_bf16(wpack)

    bpack = np.stack(
        [bq_eff.reshape(4, 128), bk_eff.reshape(4, 128),
         pb_eff.reshape(4, 128)], axis=0,
    ).reshape(12, 128).T.astype(np.float32)
    sel2d = np.zeros((2, 128), np.float32)
    sel2d[0, 0:HD] = 1.0
    sel2d[1, HD:128] = 1.0
    return (np.ascontiguousarray(wpack_bf16), np.ascontiguousarray(gsel), gselT,
            np.ascontiguousarray(bpack), sel2d)


def make_in_maps(x, norm_w, norm_b, qkv_w, qkv_b, proj_w, proj_b):
    b_sz = x.shape[0]
    wpack, gsel, gselT, bpack, sel2d = _prep_host(
        norm_w, norm_b, qkv_w, qkv_b, proj_w, proj_b
    )
    xf = np.ascontiguousarray(x.reshape(b_sz, C, N).astype(np.float32))
    return [
        {"x": xf[i], "wpack": wpack, "gsel": gsel, "gselT": gselT,
         "bpack": bpack, "sel2d": sel2d}
        for i in range(b_sz)
    ]


def kernel(x, norm_w, norm_b, qkv_w, qkv_b, proj_w, proj_b):
    x, norm_w, norm_b, qkv_w, qkv_b, proj_w, proj_b = (
        np.asarray(a, dtype=np.float32)
        for a in (x, norm_w, norm_b, qkv_w, qkv_b, proj_w, proj_b)
    )
    b_sz, c, h, w = x.shape
    assert (b_sz, c, h * w) == (8, C, N)
    nc = _get_nc()
    in_maps = make_in_maps(x, norm_w, norm_b, qkv_w, qkv_b, proj_w, proj_b)
    res = run_bass_kernel_spmd(nc, in_maps, core_ids=list(range(b_sz)))
    out = np.stack([r["y"].astype(np.float32) for r in res.results], axis=0)
    return out.reshape(b_sz, C, h, w)


# revision 13
# speedup vs baseline: 1.2769x; 1.2769x over previous
"""Trainium2 Bass kernel for nn_AttentionBlock (GroupNorm + MHA + proj + residual).

Problem: x[8, 512, 32, 32] fp32; GroupNorm(32 groups) -> qkv (1x1 conv) ->
8-head attention over 1024 spatial positions -> proj -> residual.

Sharding: data-parallel over batch. 8 batch elements -> 8 NeuronCores,
one SPMD NEFF. No collectives.

v4 design (per-core, x as [c=512, n=1024]):
  - ACT is the pacer: 64 exp chunks of [128,1024] (~67 us). All other
    ACT work moved to DVE/Pool; ALL 32 of j=0/j=1's exps start during
    the qkv phase (fp8 probs pair tiles keep 16 pairs resident).
  - PE moving-element rate is dtype-independent on this hw, so no fp8
    matmul tricks; probs/vT are fp8e4 only to shrink SBUF (probs are
    exp(s-6); the shift cancels in the softmax normalization and keeps
    exp inside fp8e4 range). AV accumulates f32 in psum, v2-shaped.
  - softmax normalization is fully on-chip (v3's DRAM round-trip
    broadcast crawled at ~2GB/s on the critical tail): denominators ->
    dd via sbuf-sbuf DMA reshape -> DVE reciprocal [128,2,8] -> rrT
    [2,N] via sbuf-sbuf DMA -> PE broadcast matmul with a 0/1
    selection lhsT [2,128] -> rb psum -> one DVE mul per j.
  - weights DMA'd in use order (q-m0, k-m0, v, q/k m1-3, proj) so the
    first score matmuls start ~15us earlier; xh standardization split
    DVE/Pool to halve its serial cost.
  - proj: kc 0-2 for all m first, then kc=3 per-m with eviction+DMA
    pipelined; y output in bf16 (halves writeback, error budget ok).
  - GroupNorm: selection-matrix matmuls for group reduce, rsqrt via
    exp(-0.5*ln(var+eps)); per-channel affine folded into qkv weights.
"""
import sys

sys.path.insert(0, "/opt/trn_rl_repo")

import numpy as np

import concourse.bass as bass
import concourse.bacc as bacc
import concourse.tile as tile
from concourse import mybir
from concourse.bass_utils import run_bass_kernel_spmd

F32 = mybir.dt.float32
F32R = mybir.dt.float32r
BF16 = mybir.dt.bfloat16
F8 = mybir.dt.float8e4
AX = mybir.AxisListType
OP = mybir.AluOpType
AF = mybir.ActivationFunctionType

C = 512          # channels
N = 1024         # spatial positions (32*32)
HEADS = 8
HD = 64          # head dim
G = 32           # groups
GSZ = 16         # channels per group
EPS = 1e-6
NC4 = 4          # channel chunks of 128
NM8 = 8          # spatial chunks of 128
EXP_SHIFT = -6.0  # exp(s-6): cancels in softmax, keeps exp in fp8e4 range
WQ0 = 0          # wpack col offsets
WK0 = 512
WV0 = 1024
WP0 = 1536
WPACK_COLS = 2048


def build_nc():
    nc = bacc.Bacc(None)
    x = nc.declare_dram_parameter("x", [C, N], F32, isOutput=False)
    wpack = nc.declare_dram_parameter("wpack", [C, WPACK_COLS], BF16, isOutput=False)
    gsel = nc.declare_dram_parameter("gsel", [C, G], F32, isOutput=False)
    gselT = nc.declare_dram_parameter("gselT", [G, C], F32, isOutput=False)
    bpack = nc.declare_dram_parameter("bpack", [128, 12], F32, isOutput=False)
    sel2d = nc.declare_dram_parameter("sel2d", [2, 128], BF16, isOutput=False)
    y = nc.declare_dram_parameter("y", [C, N], BF16, isOutput=True)

    with tile.TileContext(nc) as tc:
        with (
            tc.tile_pool(name="const", bufs=1) as const,
            tc.tile_pool(name="main", bufs=1) as main,
        ):
            # ---- Phase 0: input DMAs ----
            # x chunks on the sync queue; weight column-groups on the scalar
            # queue in USE order so q/k m=0 land first.
            x_sb = main.tile([128, NC4, N], F32)
            xv = x[:].rearrange("(c p) n -> p c n", p=128)
            for c4 in range(NC4):
                nc.sync.dma_start(x_sb[:, c4, :], xv[:, c4, :])
            gs_sb = const.tile([128, NC4, G], F32)
            nc.sync.dma_start(gs_sb[:], gsel[:].rearrange("(c p) g -> p c g", p=128))
            gt_sb = const.tile([G, C], F32)
            nc.sync.dma_start(gt_sb[:], gselT[:])
            bp_sb = const.tile([128, 12], F32)
            nc.sync.dma_start(bp_sb[:], bpack[:])
            wp_sb = const.tile([128, NC4, WPACK_COLS], BF16)
            wpv = wpack[:].rearrange("(c p) m -> p c m", p=128)
            # use-order column groups: q-m0, k-m0, v, q-m123, k-m123, proj
            for lo, hi in ((WQ0, WQ0 + 128), (WK0, WK0 + 128),
                           (WV0, WV0 + C), (WQ0 + 128, WQ0 + 512),
                           (WK0 + 128, WK0 + 512), (WP0, WP0 + C)):
                nc.scalar.dma_start(wp_sb[:, :, lo:hi], wpv[:, :, lo:hi])

            # ---- Phase 1: GroupNorm stats -> xh (standardized x, bf16) ----
            cst = const.tile([128, NC4, 2], F32)       # per-channel sum | sumsq
            gsb = const.tile([G, 8], F32)              # group scratch
            eps_t = const.tile([G, 1], F32)
            nc.vector.memset(eps_t[:], EPS)
            shift_t = const.tile([128, 1], F32)        # exp bias (-6)
            nc.vector.memset(shift_t[:], EXP_SHIFT)
            sel2 = const.tile([2, 128], BF16)          # hh-broadcast selector
            nc.sync.dma_start(sel2[:], sel2d[:])
            gstats = const.tile([G, 2], F32)           # rs | -mean*rs
            chsc = const.tile([128, NC4, 2], F32)      # per-channel rs | bias
            xh_sb = main.tile([128, NC4, N], BF16)

            # prefire the Square table set first (the stats need it next);
            # the Ln/Exp set is loaded once after the squares finish
            nc.scalar.activation(gsb[:, 4:5], eps_t[:], AF.Square)

            with (
                tc.tile_pool(name="sq", bufs=2) as sqp,
                tc.tile_pool(name="pst", bufs=2, space="PSUM") as pst,
            ):
                for c4 in range(NC4):
                    nc.vector.tensor_reduce(
                        cst[:, c4, 0:1], x_sb[:, c4, :], axis=AX.X, op=OP.add
                    )
                    sq = sqp.tile([128, N], F32)
                    nc.scalar.activation(
                        sq[:], x_sb[:, c4, :], AF.Square,
                        accum_out=cst[:, c4, 1:2],
                    )
                gs_ps = pst.tile([G, 2], F32)
                for c4 in range(NC4):
                    nc.tensor.matmul(
                        gs_ps[:],
                        gs_sb[:, c4, :],
                        cst[:, c4, :],
                        start=(c4 == 0),
                        stop=(c4 == NC4 - 1),
                    )
                nc.vector.tensor_copy(gsb[:, 0:2], gs_ps[:])
                # mean = gsb[:,0], ex2 = gsb[:,1] (both already / 16384)
                nc.vector.tensor_mul(gsb[:, 2:3], gsb[:, 0:1], gsb[:, 0:1])
                nc.vector.tensor_sub(gsb[:, 3:4], gsb[:, 1:2], gsb[:, 2:3])
                nc.scalar.activation(gsb[:, 4:5], gsb[:, 3:4], AF.Ln, bias=eps_t[:])
                nc.scalar.activation(gstats[:, 0:1], gsb[:, 4:5], AF.Exp, scale=-0.5)
                nc.vector.tensor_mul(gsb[:, 6:7], gsb[:, 0:1], gstats[:, 0:1])
                nc.vector.tensor_scalar_mul(gstats[:, 1:2], gsb[:, 6:7], -1.0)
                for c4 in range(NC4):
                    cs_ps = pst.tile([128, 2], F32)
                    nc.tensor.matmul(
                        cs_ps[:],
                        gt_sb[:, c4 * 128:(c4 + 1) * 128],
                        gstats[:],
                        start=True,
                        stop=True,
                    )
                    nc.vector.tensor_copy(chsc[:, c4, :], cs_ps[:])
                # xh standardization split DVE (c4 0,1) / Pool (c4 2,3) to
                # halve the serial chain ahead of the first qkv matmuls
                for c4 in range(NC4):
                    eng = nc.vector if c4 < 2 else nc.gpsimd
                    eng.tensor_scalar(
                        xh_sb[:, c4, :], x_sb[:, c4, :],
                        chsc[:, c4, 0:1], chsc[:, c4, 1:2],
                        op0=OP.mult, op1=OP.add,
                    )

            # ---- Phase 2: qkv ----
            q_sb = main.tile([128, NC4, N], BF16)
            k_sb = main.tile([128, NC4, N], BF16)
            vt_sb = main.tile([128, NM8, HEADS, HD + 1], F8)
            nc.vector.memset(vt_sb[:, :, :, HD:HD + 1], 1.0)

            # probs pool lives across phase 2 + 3: ALL of j=0 and j=1's exp
            # chunks run DURING the qkv phase (ACT is otherwise idle there
            # while attention is ACT-bound). Tiles are kq-PAIRS [128, 2, N]
            # in fp8e4.
            probs_pool = tc.tile_pool(name="probs", bufs=20)
            probs = probs_pool.__enter__()
            ptE = {}
            rows = (slice(0, HD), slice(HD, 128))

            with (
                tc.tile_pool(name="pmm", bufs=1, space="PSUM") as pmm,
                tc.tile_pool(name="pvv", bufs=2, space="PSUM") as pvv,
                tc.tile_pool(name="pesc", bufs=2, space="PSUM") as pesc,
            ):
                # q/k m=0 first: j=0's scores only need these
                for dst, woff, boff in ((q_sb, WQ0, 0), (k_sb, WK0, 4)):
                    ps = pmm.tile([128, N], F32, tag="qk")
                    for kc in range(NC4):
                        for nh in range(2):
                            nc.tensor.matmul(
                                ps[:, nh * 512:(nh + 1) * 512],
                                wp_sb[:, kc, woff:woff + 128],
                                xh_sb[:, kc, nh * 512:(nh + 1) * 512],
                                start=(kc == 0),
                                stop=(kc == NC4 - 1),
                            )
                    nc.vector.tensor_scalar_add(
                        dst[:, 0, :], ps[:], bp_sb[:, boff:boff + 1]
                    )
                # vT: stationary xh spatial chunk, moving all v weights
                for mt in range(NM8):
                    ps = pvv.tile([128, 512], F32, tag="v")
                    for kc in range(NC4):
                        nc.tensor.matmul(
                            ps[:],
                            xh_sb[:, kc, mt * 128:(mt + 1) * 128],
                            wp_sb[:, kc, WV0:WV0 + C],
                            start=(kc == 0),
                            stop=(kc == NC4 - 1),
                        )
                    nc.vector.tensor_copy(
                        vt_sb[:, mt, :, 0:HD],
                        ps[:].rearrange("p (h c) -> p h c", h=HEADS),
                    )

                # q/k m=1..3 emitted one matmul at a time between the early
                # score chunks below, so the PE stays dense while ACT exps
                fill = []

                def qk_unit(dst, woff, boff, m):
                    ps = pmm.tile([128, N], F32, name=f"qk{woff}_{m}", tag="qk")
                    for kc in range(NC4):
                        for nh in range(2):
                            fill.append(lambda ps=ps, kc=kc, nh=nh, woff=woff, m=m: nc.tensor.matmul(
                                ps[:, nh * 512:(nh + 1) * 512],
                                wp_sb[:, kc, woff + m * 128:woff + (m + 1) * 128],
                                xh_sb[:, kc, nh * 512:(nh + 1) * 512],
                                start=(kc == 0),
                                stop=(kc == NC4 - 1),
                            ))
                    fill.append(lambda ps=ps, dst=dst, m=m, boff=boff: nc.vector.tensor_scalar_add(
                        dst[:, m, :], ps[:], bp_sb[:, boff + m:boff + m + 1]
                    ))

                for m in range(1, 4):
                    qk_unit(q_sb, WQ0, 0, m)
                    qk_unit(k_sb, WK0, 4, m)

                # early scores+exp for ALL of j=0 and j=1 (32 chunks),
                # interleaved with the remaining qkv matmuls
                for c in range(32):
                    j01, kq, hh = c // 16, (c % 16) // 2, c % 2
                    esc = pesc.tile([128, N], F32, name=f"esc{c}", tag="esc")
                    for nh in range(2):
                        nc.tensor.matmul(
                            esc[:, nh * 512:(nh + 1) * 512],
                            k_sb[rows[hh], j01, kq * 128:(kq + 1) * 128],
                            q_sb[rows[hh], j01, nh * 512:(nh + 1) * 512],
                            start=True,
                            stop=True,
                        )
                    key = (j01, kq // 2, hh)
                    if kq % 2 == 0:
                        ptp = probs.tile([128, 2, N], F8, name=f"ptE{key}", tag="pt")
                        ptE[key] = ptp
                    else:
                        ptp = ptE[key]
                    nc.scalar.activation(
                        ptp[:, kq % 2, :], esc[:], AF.Exp, bias=shift_t[:]
                    )
                    for _ in range(2):
                        if fill:
                            fill.pop(0)()
                while fill:
                    fill.pop(0)()

            # ---- Phase 3: attention + proj ----
            # Part A (jj=0,1): chunks run hh-major; each emits scores+exp
            # for j=jj+2 plus av matmuls for j=jj (pts from phase 2), so PE
            # duty stays high through the exp-paced stretch and ALL exps
            # finish by the end of jj=1. av accumulators are per-(j,hh)
            # [65,N] (2 psum banks) and evict immediately, so the
            # normalize chain never blocks the next av allocation.
            # Part B: dense av j2/j3 + proj, PE back-to-back.
            aun_ch = main.tile([128, NC4, N], F32)      # unnormalized A
            a_sb = main.tile([128, NC4, N], BF16)       # normalized A

            with (
                tc.tile_pool(name="spool", bufs=2) as spool,
                tc.tile_pool(name="dpool", bufs=2) as dpool,
                tc.tile_pool(name="psc", bufs=2, space="PSUM") as psc,
                tc.tile_pool(name="pav", bufs=1, space="PSUM") as pav,
            ):
                def av_mms(j, kq, hh, av):
                    ptp = ptE[(j, kq // 2, hh)]
                    for nh in range(2):
                        nc.tensor.matmul(
                            av[:, nh * 512:(nh + 1) * 512],
                            vt_sb[:, kq, 2 * j + hh, :],
                            ptp[:, kq % 2, nh * 512:(nh + 1) * 512],
                            start=(kq == 0),
                            stop=(kq == NM8 - 1),
                        )

                dds = {}

                def evict_av(j, hh, av, on_act):
                    # evict av rows + the denominator row (dd, via a
                    # sbuf-sbuf DMA reshape to 128 partitions)
                    s_t = spool.tile(
                        [HD + 1, N], F32, name=f"st{j}_{hh}", tag=f"st{hh}"
                    )
                    if on_act:
                        nc.scalar.activation(s_t[:], av[:], AF.Copy)
                    else:
                        nc.vector.tensor_copy(s_t[:], av[:])
                    if hh == 0:
                        dds[j] = dpool.tile([128, 2, 8], F32, name=f"dd{j}", tag="dd")
                    nc.sync.dma_start(dds[j][:, hh, :], s_t[HD:HD + 1, :])
                    nc.sync.dma_start(aun_ch[rows[hh], j, :], s_t[0:HD, :])

                def normalize(j):
                    # fully on-chip: rr = 1/dd (DVE, bf16 out), rrT[2,N]
                    # via sbuf-sbuf DMAs, rb = sel2.T @ rrT (PE broadcast,
                    # bf16, into its own psum slot), a_sb = aun * rb (DVE)
                    rr = dpool.tile([128, 2, 8], BF16, name=f"rr{j}", tag="rr")
                    with nc.allow_low_precision(reason="bf16 softmax denom"):
                        nc.vector.reciprocal(rr[:], dds[j][:])
                    rrT = dpool.tile([2, N], BF16, name=f"rrT{j}", tag="rrT")
                    for hh in range(2):
                        nc.sync.dma_start(rrT[hh:hh + 1, :], rr[:, hh, :])
                    rb = psc.tile([128, N], F32, name=f"rb{j}", tag="rb", bufs=1)
                    for nh in range(2):
                        nc.tensor.matmul(
                            rb[:, nh * 512:(nh + 1) * 512],
                            sel2[:],
                            rrT[:, nh * 512:(nh + 1) * 512],
                            start=True,
                            stop=True,
                        )
                    nc.vector.tensor_mul(a_sb[:, j, :], aun_ch[:, j, :], rb[:])

                # ---- Part A ----
                for jj in range(2):
                    j2 = jj + 2
                    for hh in range(2):
                        av = pav.tile(
                            [HD + 1, N], F32, name=f"av{jj}_{hh}", tag="av"
                        )
                        for kq in range(NM8):
                            sps = psc.tile(
                                [128, N], F32, name=f"sps{j2}_{kq}_{hh}",
                                tag="sps",
                            )
                            # small warmth keeper every other chunk keeps
                            # the PE HAM window busy during exp-paced code
                            if kq % 2 == 0:
                                nc.tensor.matmul(
                                    sps[:, 0:128],
                                    wp_sb[:, 0, 0:128],
                                    xh_sb[:, 0, 0:128],
                                    start=True,
                                    stop=True,
                                )
                            for nh in range(2):
                                nc.tensor.matmul(
                                    sps[:, nh * 512:(nh + 1) * 512],
                                    k_sb[rows[hh], j2, kq * 128:(kq + 1) * 128],
                                    q_sb[rows[hh], j2, nh * 512:(nh + 1) * 512],
                                    start=True,
                                    stop=True,
                                )
                            key2 = (j2, kq // 2, hh)
                            if kq % 2 == 0:
                                ptp2 = probs.tile(
                                    [128, 2, N], F8,
                                    name=f"pt{j2}_{kq // 2}_{hh}", tag="pt",
                                )
                                ptE[key2] = ptp2
                            nc.scalar.activation(
                                ptE[key2][:, kq % 2, :], sps[:], AF.Exp,
                                bias=shift_t[:],
                            )
                            av_mms(jj, kq, hh, av)
                        evict_av(jj, hh, av, on_act=False)
                    normalize(jj)

                # ---- Part B: dense av j2/j3 + proj ----
                yv = y[:].rearrange("(m p) n -> m p n", p=128)

                def proj_mm(ps, m, kc, stop):
                    for nh in range(2):
                        nc.tensor.matmul(
                            ps[:, nh * 512:(nh + 1) * 512],
                            wp_sb[:, kc, WP0 + m * 128:WP0 + (m + 1) * 128],
                            a_sb[:, kc, nh * 512:(nh + 1) * 512],
                            start=(kc == 0),
                            stop=stop,
                        )

                pA = None
                for jj in (2, 3):
                    for hh in range(2):
                        av = pav.tile(
                            [HD + 1, N], F32, name=f"av{jj}_{hh}", tag="av"
                        )
                        for kq in range(NM8):
                            av_mms(jj, kq, hh, av)
                        evict_av(jj, hh, av, on_act=True)
                    normalize(jj)
                    if jj == 2:
                        # proj m=0,1 into the freed scores slots; kc=0,1
                        # overlap j2's normalize chain, kc=2 follows it
                        pA = [
                            psc.tile([128, N], F32, name=f"pjA{m}", tag="sps")
                            for m in range(2)
                        ]
                        for kc in range(2):
                            for m in range(2):
                                proj_mm(pA[m], m, kc, stop=False)
                for m in range(2):
                    proj_mm(pA[m], m, 2, stop=False)
                for m in range(2):
                    proj_mm(pA[m], m, 3, stop=True)

                with tc.tile_pool(name="ypool", bufs=2) as ypool:
                    def evict_y(src, m):
                        yt = ypool.tile([128, N], BF16, name=f"yt{m}", tag="yt")
                        # fused eviction: (psum + bias) + residual, one DVE op
                        nc.vector.scalar_tensor_tensor(
                            yt[:], src, bp_sb[:, 8 + m:9 + m], x_sb[:, m, :],
                            op0=OP.add, op1=OP.add,
                        )
                        q = (nc.sync, nc.scalar)[m % 2]
                        q.dma_start(yv[m, :, :], yt[:])

                    # m=2,3 sequentially through the single freed av slot,
                    # m=0,1 from their accumulated pA tiles
                    for m in (2, 3):
                        pB = pav.tile([128, N], F32, name=f"pjB{m}", tag="av")
                        for kc in range(NC4):
                            proj_mm(pB, m, kc, stop=(kc == NC4 - 1))
                        evict_y(pB[:], m)
                    for m in (0, 1):
                        evict_y(pA[m][:], m)

            probs_pool.__exit__(None, None, None)

    nc.compile()
    return nc


_NC_CACHE = None


def _get_nc():
    global _NC_CACHE
    if _NC_CACHE is None:
        _NC_CACHE = build_nc()
    return _NC_CACHE


def _to_bf16(a):
    import ml_dtypes
    return np.ascontiguousarray(a, np.float32).astype(ml_dtypes.bfloat16)


def _prep_host(norm_w, norm_b, qkv_w, qkv_b, proj_w, proj_b):
    g = norm_w.astype(np.float32)
    b = norm_b.astype(np.float32)
    Wq, Wk, Wv = qkv_w[0:C], qkv_w[C:2 * C], qkv_w[2 * C:3 * C]
    bq, bk, bv = qkv_b[0:C], qkv_b[C:2 * C], qkv_b[2 * C:3 * C]
    scale = np.float32(1.0 / np.sqrt(HD))

    WqT = (scale * (Wq * g[None, :])).T
    WkT = (Wk * g[None, :]).T
    WvT = (Wv * g[None, :]).T
    bq_eff = scale * (Wq @ b + bq)
    bk_eff = Wk @ b + bk
    pb_eff = proj_w @ (Wv @ b + bv) + proj_b

    cidx = np.arange(C)
    gsel = np.zeros((C, G), np.float32)
    gsel[cidx, cidx // GSZ] = np.float32(1.0 / (GSZ * N))
    gselT = np.zeros((G, C), np.float32)
    gselT[cidx // GSZ, cidx] = 1.0

    wpack = np.concatenate([WqT, WkT, WvT, proj_w.T], axis=1).astype(np.float32)
    assert wpack.shape == (C, WPACK_COLS)
    wpack_bf16 = _to_bf16(wpack)

    bpack = np.stack(
        [bq_eff.reshape(4, 128), bk_eff.reshape(4, 128),
         pb_eff.reshape(4, 128)], axis=0,
    ).reshape(12, 128).T.astype(np.float32)
    import ml_dtypes
    sel2d = np.zeros((2, 128), np.float32)
    sel2d[0, 0:HD] = 1.0
    sel2d[1, HD:128] = 1.0
    sel2d = sel2d.astype(ml_dtypes.bfloat16)
    return (np.ascontiguousarray(wpack_bf16), np.ascontiguousarray(gsel), gselT,
            np.ascontiguousarray(bpack), sel2d)


def make_in_maps(x, norm_w, norm_b, qkv_w, qkv_b, proj_w, proj_b):
    b_sz = x.shape[0]
    wpack, gsel, gselT, bpack, sel2d = _prep_host(
        norm_w, norm_b, qkv_w, qkv_b, proj_w, proj_b
    )
    xf = np.ascontiguousarray(x.reshape(b_sz, C, N).astype(np.float32))
    return [
        {"x": xf[i], "wpack": wpack, "gsel": gsel, "gselT": gselT,
         "bpack": bpack, "sel2d": sel2d}
        for i in range(b_sz)
    ]


def kernel(x, norm_w, norm_b, qkv_w, qkv_b, proj_w, proj_b):
    x, norm_w, norm_b, qkv_w, qkv_b, proj_w, proj_b = (
        np.asarray(a, dtype=np.float32)
        for a in (x, norm_w, norm_b, qkv_w, qkv_b, proj_w, proj_b)
    )
    b_sz, c, h, w = x.shape
    assert (b_sz, c, h * w) == (8, C, N)
    nc = _get_nc()
    in_maps = make_in_maps(x, norm_w, norm_b, qkv_w, qkv_b, proj_w, proj_b)
    res = run_bass_kernel_spmd(nc, in_maps, core_ids=list(range(b_sz)))
    out = np.stack([r["y"].astype(np.float32) for r in res.results], axis=0)
    return out.reshape(b_sz, C, h, w)


# revision 14
# speedup vs baseline: 1.2950x; 1.0142x over previous
"""Trainium2 Bass kernel for nn_AttentionBlock (GroupNorm + MHA + proj + residual).

Problem: x[8, 512, 32, 32] fp32; GroupNorm(32 groups) -> qkv (1x1 conv) ->
8-head attention over 1024 spatial positions -> proj -> residual.

Sharding: data-parallel over batch. 8 batch elements -> 8 NeuronCores,
one SPMD NEFF. No collectives.

v4 design (per-core, x as [c=512, n=1024]):
  - ACT is the pacer: 64 exp chunks of [128,1024] (~67 us). All other
    ACT work moved to DVE/Pool; ALL 32 of j=0/j=1's exps start during
    the qkv phase (fp8 probs pair tiles keep 16 pairs resident).
  - PE moving-element rate is dtype-independent on this hw, so no fp8
    matmul tricks; probs/vT are fp8e4 only to shrink SBUF (probs are
    exp(s-6); the shift cancels in the softmax normalization and keeps
    exp inside fp8e4 range). AV accumulates f32 in psum, v2-shaped.
  - softmax normalization is fully on-chip (v3's DRAM round-trip
    broadcast crawled at ~2GB/s on the critical tail): denominators ->
    dd via sbuf-sbuf DMA reshape -> DVE reciprocal [128,2,8] -> rrT
    [2,N] via sbuf-sbuf DMA -> PE broadcast matmul with a 0/1
    selection lhsT [2,128] -> rb psum -> one DVE mul per j.
  - weights DMA'd in use order (q-m0, k-m0, v, q/k m1-3, proj) so the
    first score matmuls start ~15us earlier; xh standardization split
    DVE/Pool to halve its serial cost.
  - proj: kc 0-2 for all m first, then kc=3 per-m with eviction+DMA
    pipelined; y output in bf16 (halves writeback, error budget ok).
  - GroupNorm: selection-matrix matmuls for group reduce, rsqrt via
    exp(-0.5*ln(var+eps)); per-channel affine folded into qkv weights.
"""
import sys

sys.path.insert(0, "/opt/trn_rl_repo")

import numpy as np

import concourse.bass as bass
import concourse.bacc as bacc
import concourse.tile as tile
from concourse import mybir
from concourse.bass_utils import run_bass_kernel_spmd

F32 = mybir.dt.float32
F32R = mybir.dt.float32r
BF16 = mybir.dt.bfloat16
F8 = mybir.dt.float8e4
AX = mybir.AxisListType
OP = mybir.AluOpType
AF = mybir.ActivationFunctionType

C = 512          # channels
N = 1024         # spatial positions (32*32)
HEADS = 8
HD = 64          # head dim
G = 32           # groups
GSZ = 16         # channels per group
EPS = 1e-6
NC4 = 4          # channel chunks of 128
NM8 = 8          # spatial chunks of 128
EXP_SHIFT = -6.0  # exp(s-6): cancels in softmax, keeps exp in fp8e4 range
WQ0 = 0          # wpack col offsets
WK0 = 512
WV0 = 1024
WP0 = 1536
WPACK_COLS = 2048


def build_nc():
    nc = bacc.Bacc(None)
    x = nc.declare_dram_parameter("x", [C, N], F32, isOutput=False)
    wpack = nc.declare_dram_parameter("wpack", [C, WPACK_COLS], BF16, isOutput=False)
    gsel = nc.declare_dram_parameter("gsel", [C, G], F32, isOutput=False)
    gselT = nc.declare_dram_parameter("gselT", [G, C], F32, isOutput=False)
    bpack = nc.declare_dram_parameter("bpack", [128, 12], F32, isOutput=False)
    sel2d = nc.declare_dram_parameter("sel2d", [2, 128], BF16, isOutput=False)
    y = nc.declare_dram_parameter("y", [128, NC4 * N], BF16, isOutput=True)

    with tile.TileContext(nc) as tc:
        with (
            tc.tile_pool(name="const", bufs=1) as const,
            tc.tile_pool(name="main", bufs=1) as main,
        ):
            # ---- Phase 0: input DMAs ----
            # x chunks on the sync queue; weight column-groups on the scalar
            # queue in USE order so q/k m=0 land first.
            x_sb = main.tile([128, NC4, N], F32)
            xv = x[:].rearrange("(c p) n -> p c n", p=128)
            for c4 in range(NC4):
                nc.sync.dma_start(x_sb[:, c4, :], xv[:, c4, :])
            gs_sb = const.tile([128, NC4, G], F32)
            nc.sync.dma_start(gs_sb[:], gsel[:].rearrange("(c p) g -> p c g", p=128))
            gt_sb = const.tile([G, C], F32)
            nc.sync.dma_start(gt_sb[:], gselT[:])
            bp_sb = const.tile([128, 12], F32)
            nc.sync.dma_start(bp_sb[:], bpack[:])
            wp_sb = const.tile([128, NC4, WPACK_COLS], BF16)
            wpv = wpack[:].rearrange("(c p) m -> p c m", p=128)
            # use-order column groups: q-m0, k-m0, v, q-m123, k-m123, proj
            for lo, hi in ((WQ0, WQ0 + 128), (WK0, WK0 + 128),
                           (WV0, WV0 + C), (WQ0 + 128, WQ0 + 512),
                           (WK0 + 128, WK0 + 512), (WP0, WP0 + C)):
                nc.scalar.dma_start(wp_sb[:, :, lo:hi], wpv[:, :, lo:hi])

            # ---- Phase 1: GroupNorm stats -> xh (standardized x, bf16) ----
            cst = const.tile([128, NC4, 2], F32)       # per-channel sum | sumsq
            gsb = const.tile([G, 8], F32)              # group scratch
            eps_t = const.tile([G, 1], F32)
            nc.vector.memset(eps_t[:], EPS)
            shift_t = const.tile([128, 1], F32)        # exp bias (-6)
            nc.vector.memset(shift_t[:], EXP_SHIFT)
            sel2 = const.tile([2, 128], BF16)          # hh-broadcast selector
            nc.sync.dma_start(sel2[:], sel2d[:])
            gstats = const.tile([G, 2], F32)           # rs | -mean*rs
            chsc = const.tile([128, NC4, 2], F32)      # per-channel rs | bias
            xh_sb = main.tile([128, NC4, N], BF16)

            # prefire the Square table set first (the stats need it next);
            # the Ln/Exp set is loaded once after the squares finish
            nc.scalar.activation(gsb[:, 4:5], eps_t[:], AF.Square)

            with (
                tc.tile_pool(name="sq", bufs=2) as sqp,
                tc.tile_pool(name="pst", bufs=2, space="PSUM") as pst,
            ):
                for c4 in range(NC4):
                    nc.vector.tensor_reduce(
                        cst[:, c4, 0:1], x_sb[:, c4, :], axis=AX.X, op=OP.add
                    )
                    sq = sqp.tile([128, N], F32)
                    nc.scalar.activation(
                        sq[:], x_sb[:, c4, :], AF.Square,
                        accum_out=cst[:, c4, 1:2],
                    )
                gs_ps = pst.tile([G, 2], F32)
                for c4 in range(NC4):
                    nc.tensor.matmul(
                        gs_ps[:],
                        gs_sb[:, c4, :],
                        cst[:, c4, :],
                        start=(c4 == 0),
                        stop=(c4 == NC4 - 1),
                    )
                nc.vector.tensor_copy(gsb[:, 0:2], gs_ps[:])
                # mean = gsb[:,0], ex2 = gsb[:,1] (both already / 16384)
                nc.vector.tensor_mul(gsb[:, 2:3], gsb[:, 0:1], gsb[:, 0:1])
                nc.vector.tensor_sub(gsb[:, 3:4], gsb[:, 1:2], gsb[:, 2:3])
                nc.scalar.activation(gsb[:, 4:5], gsb[:, 3:4], AF.Ln, bias=eps_t[:])
                nc.scalar.activation(gstats[:, 0:1], gsb[:, 4:5], AF.Exp, scale=-0.5)
                nc.vector.tensor_mul(gsb[:, 6:7], gsb[:, 0:1], gstats[:, 0:1])
                nc.vector.tensor_scalar_mul(gstats[:, 1:2], gsb[:, 6:7], -1.0)
                for c4 in range(NC4):
                    cs_ps = pst.tile([128, 2], F32)
                    nc.tensor.matmul(
                        cs_ps[:],
                        gt_sb[:, c4 * 128:(c4 + 1) * 128],
                        gstats[:],
                        start=True,
                        stop=True,
                    )
                    nc.vector.tensor_copy(chsc[:, c4, :], cs_ps[:])
                # xh standardization split DVE (c4 0,1) / Pool (c4 2,3) to
                # halve the serial chain ahead of the first qkv matmuls
                for c4 in range(NC4):
                    eng = nc.vector if c4 < 2 else nc.gpsimd
                    eng.tensor_scalar(
                        xh_sb[:, c4, :], x_sb[:, c4, :],
                        chsc[:, c4, 0:1], chsc[:, c4, 1:2],
                        op0=OP.mult, op1=OP.add,
                    )

            # ---- Phase 2: qkv ----
            q_sb = main.tile([128, NC4, N], BF16)
            k_sb = main.tile([128, NC4, N], BF16)
            vt_sb = main.tile([128, NM8, HEADS, HD + 1], F8)
            nc.vector.memset(vt_sb[:, :, :, HD:HD + 1], 1.0)

            # probs pool lives across phase 2 + 3: ALL of j=0 and j=1's exp
            # chunks run DURING the qkv phase (ACT is otherwise idle there
            # while attention is ACT-bound). Tiles are kq-PAIRS [128, 2, N]
            # in fp8e4.
            probs_pool = tc.tile_pool(name="probs", bufs=20)
            probs = probs_pool.__enter__()
            ptE = {}
            rows = (slice(0, HD), slice(HD, 128))

            with (
                tc.tile_pool(name="pmm", bufs=1, space="PSUM") as pmm,
                tc.tile_pool(name="pvv", bufs=2, space="PSUM") as pvv,
                tc.tile_pool(name="pesc", bufs=2, space="PSUM") as pesc,
            ):
                # q/k m=0 first: j=0's scores only need these
                for dst, woff, boff in ((q_sb, WQ0, 0), (k_sb, WK0, 4)):
                    ps = pmm.tile([128, N], F32, tag="qk")
                    for kc in range(NC4):
                        for nh in range(2):
                            nc.tensor.matmul(
                                ps[:, nh * 512:(nh + 1) * 512],
                                wp_sb[:, kc, woff:woff + 128],
                                xh_sb[:, kc, nh * 512:(nh + 1) * 512],
                                start=(kc == 0),
                                stop=(kc == NC4 - 1),
                            )
                    nc.vector.tensor_scalar_add(
                        dst[:, 0, :], ps[:], bp_sb[:, boff:boff + 1]
                    )
                # q/k m=1..3 and all vT units are emitted one matmul at
                # a time between the early score chunks below, so the PE
                # stays dense while ACT exps and the first exps start as
                # early as possible
                fill = []

                def v_unit(mt):
                    ps = pvv.tile([128, 512], F32, name=f"v{mt}", tag="v")
                    for kc in range(NC4):
                        fill.append(lambda ps=ps, kc=kc, mt=mt: nc.tensor.matmul(
                            ps[:],
                            xh_sb[:, kc, mt * 128:(mt + 1) * 128],
                            wp_sb[:, kc, WV0:WV0 + C],
                            start=(kc == 0),
                            stop=(kc == NC4 - 1),
                        ))
                    fill.append(lambda ps=ps, mt=mt: nc.vector.tensor_copy(
                        vt_sb[:, mt, :, 0:HD],
                        ps[:].rearrange("p (h c) -> p h c", h=HEADS),
                    ))

                def qk_unit(dst, woff, boff, m):
                    ps = pmm.tile([128, N], F32, name=f"qk{woff}_{m}", tag="qk")
                    for kc in range(NC4):
                        for nh in range(2):
                            fill.append(lambda ps=ps, kc=kc, nh=nh, woff=woff, m=m: nc.tensor.matmul(
                                ps[:, nh * 512:(nh + 1) * 512],
                                wp_sb[:, kc, woff + m * 128:woff + (m + 1) * 128],
                                xh_sb[:, kc, nh * 512:(nh + 1) * 512],
                                start=(kc == 0),
                                stop=(kc == NC4 - 1),
                            ))
                    fill.append(lambda ps=ps, dst=dst, m=m, boff=boff: nc.vector.tensor_scalar_add(
                        dst[:, m, :], ps[:], bp_sb[:, boff + m:boff + m + 1]
                    ))

                # order: q/k m=1 first (needed by the j=1 early chunks),
                # then vT (needed by part A's av j0), then q/k m=2,3
                qk_unit(q_sb, WQ0, 0, 1)
                qk_unit(k_sb, WK0, 4, 1)
                for mt in range(NM8):
                    v_unit(mt)
                for m in range(2, 4):
                    qk_unit(q_sb, WQ0, 0, m)
                    qk_unit(k_sb, WK0, 4, m)

                # early scores+exp for ALL of j=0 and j=1 (32 chunks),
                # interleaved with the remaining qkv matmuls
                for c in range(32):
                    j01, kq, hh = c // 16, (c % 16) // 2, c % 2
                    esc = pesc.tile([128, N], F32, name=f"esc{c}", tag="esc")
                    for nh in range(2):
                        nc.tensor.matmul(
                            esc[:, nh * 512:(nh + 1) * 512],
                            k_sb[rows[hh], j01, kq * 128:(kq + 1) * 128],
                            q_sb[rows[hh], j01, nh * 512:(nh + 1) * 512],
                            start=True,
                            stop=True,
                        )
                    key = (j01, kq // 2, hh)
                    if kq % 2 == 0:
                        ptp = probs.tile([128, 2, N], F8, name=f"ptE{key}", tag="pt")
                        ptE[key] = ptp
                    else:
                        ptp = ptE[key]
                    nc.scalar.activation(
                        ptp[:, kq % 2, :], esc[:], AF.Exp, bias=shift_t[:]
                    )
                    for _ in range(3):
                        if fill:
                            fill.pop(0)()
                while fill:
                    fill.pop(0)()

            # ---- Phase 3: attention + proj ----
            # Part A (jj=0,1): chunks run hh-major; each emits scores+exp
            # for j=jj+2 plus av matmuls for j=jj (pts from phase 2), so PE
            # duty stays high through the exp-paced stretch and ALL exps
            # finish by the end of jj=1. av accumulators are per-(j,hh)
            # [65,N] (2 psum banks) and evict immediately, so the
            # normalize chain never blocks the next av allocation.
            # Part B: dense av j2/j3 + proj, PE back-to-back.
            aun_ch = main.tile([128, NC4, N], F32)      # unnormalized A
            a_sb = main.tile([128, NC4, N], BF16)       # normalized A

            with (
                tc.tile_pool(name="spool", bufs=2) as spool,
                tc.tile_pool(name="dpool", bufs=2) as dpool,
                tc.tile_pool(name="psc", bufs=2, space="PSUM") as psc,
                tc.tile_pool(name="pav", bufs=1, space="PSUM") as pav,
            ):
                def av_mms(j, kq, hh, av):
                    ptp = ptE[(j, kq // 2, hh)]
                    for nh in range(2):
                        nc.tensor.matmul(
                            av[:, nh * 512:(nh + 1) * 512],
                            vt_sb[:, kq, 2 * j + hh, :],
                            ptp[:, kq % 2, nh * 512:(nh + 1) * 512],
                            start=(kq == 0),
                            stop=(kq == NM8 - 1),
                        )

                dds = {}

                def evict_av(j, hh, av, on_act):
                    # evict av rows + the denominator row (dd, via a
                    # sbuf-sbuf DMA reshape to 128 partitions)
                    s_t = spool.tile(
                        [HD + 1, N], F32, name=f"st{j}_{hh}", tag=f"st{hh}"
                    )
                    if on_act:
                        nc.scalar.activation(s_t[:], av[:], AF.Copy)
                    else:
                        nc.vector.tensor_copy(s_t[:], av[:])
                    if hh == 0:
                        dds[j] = dpool.tile([128, 2, 8], F32, name=f"dd{j}", tag="dd")
                    nc.gpsimd.dma_start(dds[j][:, hh, :], s_t[HD:HD + 1, :])
                    nc.sync.dma_start(aun_ch[rows[hh], j, :], s_t[0:HD, :])

                def normalize(j):
                    # fully on-chip: rr = 1/dd (DVE, bf16 out), rrT[2,N]
                    # via sbuf-sbuf DMAs, rb = sel2.T @ rrT (PE broadcast,
                    # bf16, into its own psum slot), a_sb = aun * rb (DVE)
                    rr = dpool.tile([128, 2, 8], BF16, name=f"rr{j}", tag="rr")
                    with nc.allow_low_precision(reason="bf16 softmax denom"):
                        nc.vector.reciprocal(rr[:], dds[j][:])
                    rrT = dpool.tile([2, N], BF16, name=f"rrT{j}", tag="rrT")
                    for hh in range(2):
                        nc.gpsimd.dma_start(rrT[hh:hh + 1, :], rr[:, hh, :])
                    rb = psc.tile([128, N], F32, name=f"rb{j}", tag="rb", bufs=1)
                    for nh in range(2):
                        nc.tensor.matmul(
                            rb[:, nh * 512:(nh + 1) * 512],
                            sel2[:],
                            rrT[:, nh * 512:(nh + 1) * 512],
                            start=True,
                            stop=True,
                        )
                    nc.vector.tensor_mul(a_sb[:, j, :], aun_ch[:, j, :], rb[:])

                # ---- Part A ----
                for jj in range(2):
                    j2 = jj + 2
                    for hh in range(2):
                        av = pav.tile(
                            [HD + 1, N], F32, name=f"av{jj}_{hh}", tag="av"
                        )
                        for kq in range(NM8):
                            sps = psc.tile(
                                [128, N], F32, name=f"sps{j2}_{kq}_{hh}",
                                tag="sps",
                            )
                            for nh in range(2):
                                nc.tensor.matmul(
                                    sps[:, nh * 512:(nh + 1) * 512],
                                    k_sb[rows[hh], j2, kq * 128:(kq + 1) * 128],
                                    q_sb[rows[hh], j2, nh * 512:(nh + 1) * 512],
                                    start=True,
                                    stop=True,
                                )
                            key2 = (j2, kq // 2, hh)
                            if kq % 2 == 0:
                                ptp2 = probs.tile(
                                    [128, 2, N], F8,
                                    name=f"pt{j2}_{kq // 2}_{hh}", tag="pt",
                                )
                                ptE[key2] = ptp2
                            nc.scalar.activation(
                                ptE[key2][:, kq % 2, :], sps[:], AF.Exp,
                                bias=shift_t[:],
                            )
                            av_mms(jj, kq, hh, av)
                        evict_av(jj, hh, av, on_act=False)
                    normalize(jj)

                # ---- Part B: dense av j2/j3 + proj ----
                yv = y[:].rearrange("(m p) n -> m p n", p=128)

                def proj_mm(ps, m, kc, stop):
                    for nh in range(2):
                        nc.tensor.matmul(
                            ps[:, nh * 512:(nh + 1) * 512],
                            wp_sb[:, kc, WP0 + m * 128:WP0 + (m + 1) * 128],
                            a_sb[:, kc, nh * 512:(nh + 1) * 512],
                            start=(kc == 0),
                            stop=stop,
                        )

                # av j2: dense, then its normalize overlaps proj m=0,1
                # kc=0-2; av j3 dense; its normalize overlaps proj m=2
                # kc=0-2; the post-mul-j3 chain is just kc=3 + evictions.
                for hh in range(2):
                    av = pav.tile([HD + 1, N], F32, name=f"av2_{hh}", tag="av")
                    for kq in range(NM8):
                        av_mms(2, kq, hh, av)
                    evict_av(2, hh, av, on_act=True)
                normalize(2)
                pA = [
                    psc.tile([128, N], F32, name=f"pjA{m}", tag="sps")
                    for m in range(2)
                ]
                for kc in range(2):
                    for m in range(2):
                        proj_mm(pA[m], m, kc, stop=False)
                for hh in range(2):
                    av = pav.tile([HD + 1, N], F32, name=f"av3_{hh}", tag="av")
                    for kq in range(NM8):
                        av_mms(3, kq, hh, av)
                    evict_av(3, hh, av, on_act=True)
                # pB m=2 kc=0-2 runs during j3's normalize chain
                pB2 = pav.tile([128, N], F32, name="pjB2", tag="av")
                for kc in range(3):
                    proj_mm(pB2, 2, kc, stop=False)
                for m in range(2):
                    proj_mm(pA[m], m, 2, stop=False)
                normalize(3)
                for m in range(2):
                    proj_mm(pA[m], m, 3, stop=True)
                proj_mm(pB2, 2, 3, stop=True)

                with tc.tile_pool(name="ypool", bufs=1) as ypool:
                    yt = ypool.tile([128, NC4, N], BF16, name="yt", tag="yt")

                    def evict_y(src, m):
                        # fused eviction: (psum + bias) + residual, one DVE op
                        nc.vector.scalar_tensor_tensor(
                            yt[:, m, :], src, bp_sb[:, 8 + m:9 + m],
                            x_sb[:, m, :],
                            op0=OP.add, op1=OP.add,
                        )

                    for m in (0, 1):
                        evict_y(pA[m][:], m)
                    nc.sync.dma_start(y[:, 0:2 * N], yt[:, 0:2, :])
                    evict_y(pB2[:], 2)
                    pB3 = pav.tile([128, N], F32, name="pjB3", tag="av")
                    for kc in range(NC4):
                        proj_mm(pB3, 3, kc, stop=(kc == NC4 - 1))
                    evict_y(pB3[:], 3)
                    nc.scalar.dma_start(y[:, 2 * N:4 * N], yt[:, 2:4, :])

            probs_pool.__exit__(None, None, None)

    nc.compile()
    return nc


_NC_CACHE = None


def _get_nc():
    global _NC_CACHE
    if _NC_CACHE is None:
        _NC_CACHE = build_nc()
    return _NC_CACHE


def _to_bf16(a):
    import ml_dtypes
    return np.ascontiguousarray(a, np.float32).astype(ml_dtypes.bfloat16)


def _prep_host(norm_w, norm_b, qkv_w, qkv_b, proj_w, proj_b):
    g = norm_w.astype(np.float32)
    b = norm_b.astype(np.float32)
    Wq, Wk, Wv = qkv_w[0:C], qkv_w[C:2 * C], qkv_w[2 * C:3 * C]
    bq, bk, bv = qkv_b[0:C], qkv_b[C:2 * C], qkv_b[2 * C:3 * C]
    scale = np.float32(1.0 / np.sqrt(HD))

    WqT = (scale * (Wq * g[None, :])).T
    WkT = (Wk * g[None, :]).T
    WvT = (Wv * g[None, :]).T
    bq_eff = scale * (Wq @ b + bq)
    bk_eff = Wk @ b + bk
    pb_eff = proj_w @ (Wv @ b + bv) + proj_b

    cidx = np.arange(C)
    gsel = np.zeros((C, G), np.float32)
    gsel[cidx, cidx // GSZ] = np.float32(1.0 / (GSZ * N))
    gselT = np.zeros((G, C), np.float32)
    gselT[cidx // GSZ, cidx] = 1.0

    wpack = np.concatenate([WqT, WkT, WvT, proj_w.T], axis=1).astype(np.float32)
    assert wpack.shape == (C, WPACK_COLS)
    wpack_bf16 = _to_bf16(wpack)

    bpack = np.stack(
        [bq_eff.reshape(4, 128), bk_eff.reshape(4, 128),
         pb_eff.reshape(4, 128)], axis=0,
    ).reshape(12, 128).T.astype(np.float32)
    import ml_dtypes
    sel2d = np.zeros((2, 128), np.float32)
    sel2d[0, 0:HD] = 1.0
    sel2d[1, HD:128] = 1.0
    sel2d = sel2d.astype(ml_dtypes.bfloat16)
    return (np.ascontiguousarray(wpack_bf16), np.ascontiguousarray(gsel), gselT,
            np.ascontiguousarray(bpack), sel2d)


def make_in_maps(x, norm_w, norm_b, qkv_w, qkv_b, proj_w, proj_b):
    b_sz = x.shape[0]
    wpack, gsel, gselT, bpack, sel2d = _prep_host(
        norm_w, norm_b, qkv_w, qkv_b, proj_w, proj_b
    )
    xf = np.ascontiguousarray(x.reshape(b_sz, C, N).astype(np.float32))
    return [
        {"x": xf[i], "wpack": wpack, "gsel": gsel, "gselT": gselT,
         "bpack": bpack, "sel2d": sel2d}
        for i in range(b_sz)
    ]


def kernel(x, norm_w, norm_b, qkv_w, qkv_b, proj_w, proj_b):
    x, norm_w, norm_b, qkv_w, qkv_b, proj_w, proj_b = (
        np.asarray(a, dtype=np.float32)
        for a in (x, norm_w, norm_b, qkv_w, qkv_b, proj_w, proj_b)
    )
    b_sz, c, h, w = x.shape
    assert (b_sz, c, h * w) == (8, C, N)
    nc = _get_nc()
    in_maps = make_in_maps(x, norm_w, norm_b, qkv_w, qkv_b, proj_w, proj_b)
    res = run_bass_kernel_spmd(nc, in_maps, core_ids=list(range(b_sz)))
    outs = []
    for r in res.results:
        yd = r["y"].astype(np.float32).reshape(128, NC4, N)
        outs.append(np.transpose(yd, (1, 0, 2)).reshape(C, N))
    return np.stack(outs, axis=0).reshape(b_sz, C, h, w)
